# revision 81
# baseline (speedup 1.0000x reference)
"""Trainium2 Bass kernel for nn_LocalAttention (B=4, L=2048, D=512, H=8).

Sharding: 8 cores = batch (4) x head-group (2). Core c handles batch c//2,
heads [4g, 4g+4) where g = c%2; host sums the two partial out-projections.

Design (cost-model-driven):
  - All matmuls run at 1 cycle/row (bf16 moving operand) or 0.5 (fp8
    DoubleRow for QK^T scores, head-dim split 32+32 across the two DR
    k-tiles).
  - Scores computed transposed S_T[n, l] so exp output e[n, l] feeds the
    attention*V matmul as lhsT directly; AV output is out^T [l, 65] with a
    ones-column in vT producing the softmax denominator for free (the
    output free-dim is 65 instead of 512, 8x fewer PE rows per call).
  - exp is the hard bottleneck (Act engine ~0.83 ns/elem, 16.8M elems per
    core): split per n-tile between Act (table exp), DVE and DVE+Pool
    (nested-quartic polynomial ((a*s+b)^2+g)^2, fp16 intermediates).
  - q/k/v biases folded into the projection matmuls via K=1 ones-row calls.
  - GPSIMD cannot touch PSUM, so PSUM evictions live on Act (q/k/vT) and
    DVE (O_un, transposes, y); Pool gets SBUF-only poly stages + norm mults.
  - AV accumulates 4 l-tiles into one PSUM bank (one zero-region
    accumulation group), evicted in one DVE instr.
  - out^T -> O via PE transpose (identity), normalization via per-partition
    reciprocal+tensor_scalar in the [l, c] layout before transposing.
"""
import os

os.environ.setdefault("MYCRO_LOCAL_CACHE", "1")

import numpy as np
import ml_dtypes
import concourse.bass as bass
import concourse.mybir as mybir
import concourse.tile as tile
from concourse.bass_utils import run_bass_kernel_spmd

F32 = mybir.dt.float32
BF16 = mybir.dt.bfloat16
FP16 = mybir.dt.float16
FP8 = mybir.dt.float8e4
AF = mybir.ActivationFunctionType
ALU = mybir.AluOpType
DR = mybir.MatmulPerfMode.DoubleRow

# ---- knobs ----
USE_FP8_SCORES = True
# exp n-tile split per (h, lc) unit: (pool_j, dve_j) -> first pool_j tiles
# to DVE(t,u)+Pool(w,e), next dve_j full-DVE, rest to Act.
UNIT_SPLITS = [
    (3, 1), (3, 1), (3, 1), (3, 1),
    (3, 1), (3, 1), (3, 1), (3, 2),
]
# nested-quartic exp fit on [-1.55, 1.55]: e^s ~ ((A*s+B)^2 + G)^2
PA, PB, PG = 0.3497715, 0.76903042, 0.41844475
# q/k weights are pre-scaled by 16 (fp8 subnormal avoidance); raw scores
# carry 16*16 on top of the reference's 1/sqrt(64) scaling
SLAM = 1.0 / (16.0 * 16.0 * 8.0)

# The walrus build in this container rejects >2 sync waits on one CTRL
# instruction; split the TileContext tail-drain's global-clock waits across
# single-wait drains.
_orig_drain = tile.TileContext._drain_and_barrier


def _patched_drain(self, tick_clock, wait_clock):
    probe = self.nc.sync.drain()
    wait_clock.add_sem_waits(
        probe.ins, tile.ScopedClock({None: tick_clock.global_clock})
    )
    si = probe.ins.sync_info
    waits = list(si.on_wait or [])
    if len(waits) > 1:
        si.on_wait = waits[:1]
        for w in waits[1:]:
            extra = self.nc.sync.drain()
            extra.ins.sync_info = mybir.SyncInfo(on_wait=[w], on_update=[])
    self.nc.all_engine_barrier()
    popped = self.nc._tile_sem_poison_stack.pop()
    assert popped is self._sem_poison
    self.nc.clear_and_free_semaphores(list(self.sems.allocated().values()))
    self.nc.all_engine_barrier()


tile.TileContext._drain_and_barrier = _patched_drain

MAX_WAITS = 1


def _split_waits(nc):
    """Hoist excess sem-waits onto same-engine nops inserted before the
    instruction (this walrus rejects >2 sync waits per instruction)."""
    for bb in nc.main_func.blocks:
        insts = bb.instructions
        i = 0
        while i < len(insts):
            ins = insts[i]
            si = ins.sync_info
            if si is not None and si.on_wait and len(si.on_wait) > MAX_WAITS:
                waits = list(si.on_wait)
                si.on_wait = waits[-MAX_WAITS:]
                extra = waits[:-MAX_WAITS]
                pos = i
                for j in range(0, len(extra), MAX_WAITS):
                    nop = nc.engines[ins.engine].nop()
                    nop_ins = nop.ins
                    for src_bb in nc.main_func.blocks:
                        if src_bb.instructions and src_bb.instructions[-1] is nop_ins:
                            src_bb.instructions.pop()
                            break
                    nop_ins.sync_info = mybir.SyncInfo(
                        on_wait=extra[j:j + MAX_WAITS], on_update=[]
                    )
                    insts.insert(pos, nop_ins)
                    pos += 1
                    i += 1
            i += 1


L = 2048
QK_DT = FP8 if USE_FP8_SCORES else BF16

LAST_RESULTS = None
_NC = None


def _build():
    nc = bass.Bass()
    xq_d = nc.dram_tensor("xq", [4, 128, L], FP8, kind="ExternalInput")
    xk_d = nc.dram_tensor("xk", [4, 128, L], FP8, kind="ExternalInput")
    xv_d = nc.dram_tensor("xv", [4, 128, L], BF16, kind="ExternalInput")
    wq_d = nc.dram_tensor("wq", [2, 128, 2, 2, 128], FP8,
                          kind="ExternalInput")
    wk_d = nc.dram_tensor("wk", [2, 128, 2, 2, 128], FP8,
                          kind="ExternalInput")
    wv_d = nc.dram_tensor("wv", [4, 128, 256], BF16, kind="ExternalInput")
    wo_d = nc.dram_tensor("wo", [2, 128, 512], BF16, kind="ExternalInput")
    bq_d = nc.dram_tensor("bq", [1, 256], BF16, kind="ExternalInput")
    bk_d = nc.dram_tensor("bk", [1, 256], BF16, kind="ExternalInput")
    bv_d = nc.dram_tensor("bv", [1, 256], BF16, kind="ExternalInput")
    bo_d = nc.dram_tensor("bo", [4, 128, 1], F32, kind="ExternalInput")
    id_d = nc.dram_tensor("ident", [128, 128], BF16, kind="ExternalInput")
    out_d = nc.dram_tensor("out", [512, L], BF16, kind="ExternalOutput")

    with tile.TileContext(nc) as tc:
        with (
            nc.allow_low_precision(reason="bf16/fp8 compute by design"),
            tc.tile_pool(name="wp", bufs=1) as wp,      # weights/persistent
            tc.tile_pool(name="per", bufs=1) as per,    # big persistent acts
            tc.tile_pool(name="stg", bufs=3) as stg,    # x staging
            tc.tile_pool(name="eb", bufs=2) as eb,      # poly-e ping-pong
            tc.tile_pool(name="eba", bufs=3) as eba,    # act-e triple buffer
            tc.tile_pool(name="sc", bufs=2) as scp,     # poly scratch
            tc.tile_pool(name="ev", bufs=6) as ev,      # small evict staging
            tc.tile_pool(name="ps_s", bufs=2, space="PSUM") as ps_s,
            tc.tile_pool(name="ps_p", bufs=1, space="PSUM") as ps_p,
            tc.tile_pool(name="ps_u", bufs=2, space="PSUM") as ps_u,
        ):
            # ---------------- persistent tiles ----------------
            wq_t = [wp.tile([128, 2, 2, 128], FP8, tag=f"wq{i}",
                            name=f"wq{i}") for i in range(2)]
            wk_t = [wp.tile([128, 2, 2, 128], FP8, tag=f"wk{i}",
                            name=f"wk{i}") for i in range(2)]
            wv_t = [wp.tile([128, 256], BF16, tag=f"wv{i}", name=f"wv{i}")
                    for i in range(4)]
            wo_t = [wp.tile([128, 512], BF16, tag=f"wo{i}", name=f"wo{i}")
                    for i in range(2)]
            bq_t = wp.tile([1, 256], BF16, tag="bq", name="bq")
            bk_t = wp.tile([1, 256], BF16, tag="bk", name="bk")
            bv_t = wp.tile([1, 256], BF16, tag="bv", name="bv")
            bo_t = [wp.tile([128, 1], F32, tag=f"bo{i}", name=f"bo{i}")
                    for i in range(4)]
            id_t = wp.tile([128, 128], BF16, tag="id", name="id")
            ones_t = wp.tile([1, 512], BF16, tag="ones", name="ones")

            xv_t = [per.tile([128, L], BF16, tag=f"xv{i}", name=f"xv{i}")
                    for i in range(4)]
            qh_t = per.tile([128, 2, L], QK_DT, tag="qh", name="qh")
            kh_t = per.tile([128, 2, L], QK_DT, tag="kh", name="kh")
            vT_t = per.tile([128, 16, 4, 65], BF16, tag="vT", name="vT")
            oun_t = per.tile([128, 16, 4, 65], BF16, tag="oun", name="oun")
            onrm_t = per.tile([128, 16, 256], BF16, tag="onrm", name="onrm")
            o_t = [per.tile([128, L], BF16, tag=f"o{i}", name=f"o{i}")
                   for i in range(2)]
            rcp_t = per.tile([128, 16, 4], F32, tag="rcp", name="rcp")

            def late_dmas():
                # On the Act DGE: keeps the SP queues free for the xk/xq
                # stage transfers that gate unit-0 scores.
                for i in range(4):
                    nc.scalar.dma_start(wv_t[i][:], wv_d[i])
                    nc.scalar.dma_start(xv_t[i][:], xv_d[i])
                nc.scalar.dma_start(bv_t[:], bv_d[:])
                for i in range(2):
                    nc.scalar.dma_start(wo_t[i][:], wo_d[i])
                for i in range(4):
                    nc.scalar.dma_start(bo_t[i][:], bo_d[i])
                nc.scalar.dma_start(id_t[:], id_d[:])

            # ---------------- projection emitters ----------------
            def stage_x(x_d, lc, dma_eng=None):
                xs = stg.tile([128, 4, 512], FP8, tag="xs", name="xs")
                (dma_eng or nc.sync).dma_start(
                    xs[:],
                    x_d[:, :, lc * 512:(lc + 1) * 512].rearrange(
                        "k p l -> p k l"
                    ),
                )
                return xs

            def qk_proj_lc(x_d, w_t, b_t, dst, lc, dma_eng=None,
                           evict_eng=None, xs=None):
                """One 512-wide l-chunk of the q/k projection, fp8
                DoubleRow over kt pairs."""
                if xs is None:
                    xs = stage_x(x_d, lc, dma_eng)
                ps = ps_s.tile([128, 1024], F32, tag="s", name="s")
                for s in range(2):
                    for kp in range(2):
                        nc.tensor.matmul(
                            ps[:, s * 512:(s + 1) * 512],
                            w_t[kp][:, :, s, :],
                            xs[:, 2 * kp:2 * kp + 2, :],
                            start=(kp == 0),
                            stop=False,
                            perf_mode=DR,
                        )
                    nc.tensor.matmul(
                        ps[:, s * 512:(s + 1) * 512],
                        b_t[0:1, s * 128:(s + 1) * 128],
                        ones_t[0:1, 0:512],
                        start=False,
                        stop=True,
                    )
                if evict_eng is None:
                    nc.scalar.copy(
                        dst[:, :, lc * 512:(lc + 1) * 512],
                        ps[:].rearrange("p (s f) -> p s f", s=2),
                    )
                else:
                    evict_eng.tensor_copy(
                        dst[:, :, lc * 512:(lc + 1) * 512],
                        ps[:].rearrange("p (s f) -> p s f", s=2),
                    )

            def v_proj_lt(lt):
                """One 128-wide l(=n)-tile of the transposed v projection."""
                ps = ps_s.tile([128, 1024], F32, tag="s", name="s")
                for kt in range(4):
                    nc.tensor.matmul(
                        ps[:, 0:256],
                        xv_t[kt][:, lt * 128:(lt + 1) * 128],
                        wv_t[kt][:],
                        start=(kt == 0),
                        stop=False,
                    )
                nc.tensor.matmul(
                    ps[:, 0:256],
                    ones_t[0:1, 0:128],
                    bv_t[:],
                    start=False,
                    stop=True,
                )
                nc.scalar.copy(
                    vT_t[:, lt, :, 0:64],
                    ps[:, 0:256].rearrange("p (h d) -> p h d", h=4),
                )

            # ---------------- attention unit pieces ----------------
            def emit_scores(h, lc, j, pool=None):
                ps = (pool or ps_s).tile([128, 1024], F32, tag="s", name="s")
                p0 = 32 * h
                if USE_FP8_SCORES:
                    for c in range(2):
                        nc.tensor.matmul(
                            ps[:, c * 512:(c + 1) * 512],
                            kh_t[p0:p0 + 32, :, j * 128:(j + 1) * 128],
                            qh_t[p0:p0 + 32, :,
                                 lc * 1024 + c * 512:lc * 1024 + (c + 1) * 512],
                            start=True,
                            stop=True,
                            perf_mode=DR,
                            tile_position=(p0, 0),
                        )
                else:
                    for c in range(2):
                        for s in range(2):
                            nc.tensor.matmul(
                                ps[:, c * 512:(c + 1) * 512],
                                kh_t[p0:p0 + 32, s, j * 128:(j + 1) * 128],
                                qh_t[p0:p0 + 32, s,
                                     lc * 1024 + c * 512:lc * 1024 + (c + 1) * 512],
                                start=(s == 0),
                                stop=(s == 1),
                                tile_position=(p0, 0),
                            )
                return ps

            def emit_exp(ps, e_a, e_p, j, pool_j, dve_j):
                """exp of one [128, 1024] score tile. Poly tiles sit at HIGH
                j slots (so the next unit's first AV chunks depend only on
                Act tiles) and live in a SEPARATE tile (e_p) so Act's exp
                stream never chains behind DVE/Pool via tile-granular WAW
                tracking. They are also EMITTED first so the DVE/Pool chains
                overlap Act's stream."""
                dst = e_a[:, j, :] if j < 11 else e_p[:, j - 11, :]
                if j >= 16 - pool_j:
                    # DVE computes t, t^2, +G; Pool squares into e
                    t = scp.tile([128, 1024], FP16, tag="t", name="t")
                    nc.vector.tensor_scalar(
                        t[:], ps[:], PA * SLAM, PB, op0=ALU.mult, op1=ALU.add
                    )
                    u = scp.tile([128, 1024], FP16, tag="u", name="u")
                    w = scp.tile([128, 1024], FP16, tag="w", name="w")
                    nc.vector.tensor_tensor(u[:], t[:], t[:], ALU.mult)
                    nc.vector.tensor_scalar_add(w[:], u[:], PG)
                    nc.gpsimd.tensor_tensor(dst, w[:], w[:], ALU.mult)
                elif j >= 16 - pool_j - dve_j:
                    t = scp.tile([128, 1024], FP16, tag="t", name="t")
                    nc.vector.tensor_scalar(
                        t[:], ps[:], PA * SLAM, PB, op0=ALU.mult, op1=ALU.add
                    )
                    u = scp.tile([128, 1024], FP16, tag="u", name="u")
                    w = scp.tile([128, 1024], FP16, tag="w", name="w")
                    nc.vector.tensor_tensor(u[:], t[:], t[:], ALU.mult)
                    nc.vector.tensor_scalar_add(w[:], u[:], PG)
                    nc.vector.tensor_tensor(dst, w[:], w[:], ALU.mult)
                else:
                    nc.scalar.activation(dst, ps[:], AF.Exp, scale=SLAM)

            def av_chunks(h, lc, e_a, e_p):
                """Deferred AV emission for one (h, lc) unit: 16 chunks of 8
                matmuls; 4 l-tiles accumulate into one PSUM bank group."""
                out = []
                tiles = {}

                def chunk(lt, jh):
                    def go():
                        g, lti = divmod(lt, 4)
                        if (lt, jh) == (g * 4, 0):
                            tiles[g] = ps_u.tile([128, 512], F32, tag="u",
                                                 name="u")
                        ut = tiles[g]
                        for j in range(jh * 8, jh * 8 + 8):
                            esrc = (e_a[:, j, lt * 128:(lt + 1) * 128]
                                    if j < 11 else
                                    e_p[:, j - 11, lt * 128:(lt + 1) * 128])
                            nc.tensor.matmul(
                                ut[:, lti * 65:lti * 65 + 65],
                                esrc,
                                vT_t[:, j, h, :],
                                start=(lti == 0 and j == 0),
                                stop=(lti == 3 and j == 15),
                            )
                        if lti == 3 and jh == 1:
                            # evict 4 l-tiles' out^T (+rowsum col) to SBUF
                            nc.vector.tensor_copy(
                                oun_t[:, lc * 8 + 4 * g:lc * 8 + 4 * g + 4,
                                      h, :],
                                ut[:, 0:260].rearrange(
                                    "p (l d) -> p l d", d=65
                                ),
                            )
                    return go

                # jh-major: chunks needing only j<8 of the previous unit
                # first, so late Pool/DVE e-tiles (written last there) are
                # not waited on at the head of the PE queue.
                for jh in range(2):
                    for lt in range(8):
                        out.append(chunk(lt, jh))
                return out

            # lt here is the GLOBAL l-tile index 0..15
            def norm_transpose_lt(lt):
                def go():
                    nc.vector.reciprocal(
                        rcp_t[:, lt, :], oun_t[:, lt, :, 64:65].rearrange(
                            "p h d -> p (h d)"
                        )
                    )
                    # mid-stream (lc0) norm mults on DVE (4x mode, cheap);
                    # epilogue (lc1) on Pool, which is idle there
                    mul_eng = nc.vector if lt < 8 else nc.gpsimd
                    for h in range(4):
                        mul_eng.tensor_scalar_mul(
                            onrm_t[:, lt, h * 64:(h + 1) * 64],
                            oun_t[:, lt, h, 0:64],
                            rcp_t[:, lt, h:h + 1],
                        )
                    for ct in range(2):
                        ut = ps_u.tile([128, 512], F32, tag="u", name="u")
                        utb = ut[:].bitcast(BF16)
                        nc.tensor.transpose(
                            utb[:, 0:128],
                            onrm_t[:, lt, ct * 128:(ct + 1) * 128],
                            id_t[:],
                        )
                        nc.vector.tensor_copy(
                            o_t[ct][:, lt * 128:(lt + 1) * 128], utb[:, 0:128]
                        )
                return go

            def out_proj(ot, qc):
                """One 512-col quarter of the out-projection: depends only
                on l-tiles 4qc..4qc+3 being transposed into O."""
                def go():
                    ps = ps_s.tile([128, 1024], F32, tag="s", name="s")
                    for ct in range(2):
                        nc.tensor.matmul(
                            ps[:, 0:512],
                            wo_t[ct][:, ot * 128:(ot + 1) * 128],
                            o_t[ct][:, qc * 512:(qc + 1) * 512],
                            start=(ct == 0),
                            stop=(ct == 1),
                        )
                    ob = ev.tile([128, 512], BF16, tag="ob", name="ob")
                    if qc < 2:
                        # mid-stream: DVE evict (Act is busy with exps)
                        nc.vector.tensor_scalar_add(
                            ob[:], ps[:, 0:512], bo_t[ot][:, 0:1]
                        )
                    else:
                        # epilogue: Act is idle there
                        nc.scalar.activation(
                            ob[:], ps[:, 0:512], AF.Identity,
                            bias=bo_t[ot][:, 0:1]
                        )
                    nc.sync.dma_start(
                        out_d[ot * 128:(ot + 1) * 128,
                              qc * 512:(qc + 1) * 512],
                        ob[:],
                    )
                return go

            # ---------------- prologue ----------------
            # The first xk chunk is the biggest critical transfer: issue it
            # before everything else; k-proj weights follow on SP while the
            # q-side staging goes through the Act DGE in parallel.
            xs_k0 = stage_x(xk_d, 0)
            for i in range(2):
                nc.sync.dma_start(wk_t[i][:], wk_d[i])
            nc.sync.dma_start(bk_t[:], bk_d[:])
            nc.vector.memset(ones_t[:], 1.0)
            # vT ones columns (rowsum trick)
            nc.vector.memset(vT_t[:, :, :, 64:65], 1.0)
            # Minimal critical path before the first scores: k lc0 + q lc0/1.
            qk_proj_lc(xk_d, wk_t, bk_t, kh_t, 0, xs=xs_k0)
            for i in range(2):
                nc.sync.dma_start(wq_t[i][:], wq_d[i])
            nc.sync.dma_start(bq_t[:], bq_d[:])
            for lc in range(2):
                qk_proj_lc(xq_d, wq_t, bq_t, qh_t, lc,
                           dma_eng=nc.scalar, evict_eng=nc.vector)
            late_dmas()

            # extras drained inside the unit j-loops (small emission chunks).
            # k lc1..3 must land before scores j=4/8/12 of unit 0; vT before
            # the AV chunks of unit 1.
            xs_k1 = stage_x(xk_d, 1)
            extras = []
            extras += [lambda: qk_proj_lc(xk_d, wk_t, bk_t, kh_t, 1,
                                          xs=xs_k1)]
            extras += [lambda lc=lc: qk_proj_lc(xk_d, wk_t, bk_t, kh_t, lc)
                       for lc in (2, 3)]
            extras += [lambda lt=lt: v_proj_lt(lt) for lt in range(16)]
            extras += [lambda lc=lc: qk_proj_lc(
                xq_d, wq_t, bq_t, qh_t, lc,
                dma_eng=nc.scalar, evict_eng=nc.vector) for lc in (2, 3)]

            units = [(lc, h) for lc in range(2) for h in range(4)]
            prev_av = []
            for ui, (lc, h) in enumerate(units):
                e_a = eba.tile([128, 11, 1024], BF16, tag="ea", name="ea")
                e_p = eb.tile([128, 5, 1024], BF16, tag="ep", name="ep")
                pool_j, dve_j = UNIT_SPLITS[ui]
                pending = list(prev_av)
                # Poly slots (high j) interleaved 1:1 with Act slots at the
                # unit front: Act's first exp starts immediately while the
                # DVE/Pool chains ramp. Unit 0 must stay ascending: its
                # scores chase the k-proj chunks arriving via extras.
                if ui == 0:
                    j_order = list(range(16))
                else:
                    polys = list(range(16 - pool_j - dve_j, 16))
                    acts = list(range(16 - pool_j - dve_j))
                    j_order = []
                    for i in range(16):
                        if i % 2 == 0 and polys:
                            j_order.append(polys.pop(0))
                        elif acts:
                            j_order.append(acts.pop(0))
                        elif polys:
                            j_order.append(polys.pop(0))
                for si, j in enumerate(j_order):
                    is_poly = j >= 16 - pool_j - dve_j
                    ps = emit_scores(h, lc, j,
                                     pool=ps_p if is_poly else ps_s)
                    emit_exp(ps, e_a, e_p, j, pool_j, dve_j)
                    if extras and (ui <= 1 or si % 3 == 2):
                        extras.pop(0)()
                    # consume AV chunks at 2/slot so the previous unit's
                    # e-tiles are released by mid-unit
                    for _ in range(2):
                        if pending:
                            pending.pop(0)()
                for fn in pending:
                    fn()
                prev_av = av_chunks(h, lc, e_a, e_p)
                if ui == 4:
                    # lc0's AV fully evicted after unit (lc1, h0): queue
                    # lc0 normalize/transpose + lch=0 out-projection,
                    # quarter-granular so each depends on 4 l-tiles only
                    extras += [norm_transpose_lt(lt) for lt in range(4)]
                    extras += [out_proj(ot, 0) for ot in range(4)]
                    extras += [norm_transpose_lt(lt) for lt in range(4, 8)]
                    extras += [out_proj(ot, 1) for ot in range(4)]

            # ---------------- epilogue ----------------
            tail = list(extras)
            tail += [norm_transpose_lt(lt) for lt in range(8, 12)]
            tail += [out_proj(ot, 2) for ot in range(4)]
            tail += [norm_transpose_lt(lt) for lt in range(12, 16)]
            tail += [out_proj(ot, 3) for ot in range(4)]
            for i, fn in enumerate(prev_av):
                fn()
                if i >= 11 and tail:
                    tail.pop(0)()
            for fn in tail:
                fn()
    _split_waits(nc)
    return nc


def kernel(q, k, v, input_mask, Wq, bq, Wk, bk, Wv, bv, Wout, bout):
    global _NC, LAST_RESULTS
    q = np.asarray(q, np.float32)
    k = np.asarray(k, np.float32)
    v = np.asarray(v, np.float32)
    Wq = np.asarray(Wq, np.float32)
    Wk = np.asarray(Wk, np.float32)
    Wv = np.asarray(Wv, np.float32)
    Wout = np.asarray(Wout, np.float32)
    bq = np.asarray(bq, np.float32)
    bk = np.asarray(bk, np.float32)
    bv = np.asarray(bv, np.float32)
    bout = np.asarray(bout, np.float32)

    if _NC is None:
        _NC = _build()

    bf = ml_dtypes.bfloat16
    f8 = ml_dtypes.float8_e4m3fn
    scale = 1.0 / 8.0  # 1/sqrt(head_dim)

    def bfv(a):
        return np.ascontiguousarray(a.astype(bf)).view(np.uint16)

    def f8v(a):
        return np.ascontiguousarray(a.astype(f8)).view(np.uint8)

    in_maps = []
    for c in range(8):
        b, g = divmod(c, 2)
        sl = slice(g * 256, (g + 1) * 256)
        # slot layout: partition p = 32h+i <-> head h dim d = 32s+i
        # out-channel = 256g + 64h + 32s + i
        # x16: lift fp8 weights out of the e4m3 subnormal range; the
        # combined 16*16/8 is divided back out in the exp scale (SLAM)
        wq_sl = Wq[sl] * 16.0      # [256, 512]
        wk_sl = Wk[sl] * 16.0
        # [out_ch 256, in 512] -> DoubleRow pairs over kt:
        # [kp 2, p 128, e 2, s 2, col 128]
        def prep_qk(w):
            w4 = w.reshape(4, 2, 32, 512)       # h, s, i, in
            w4 = w4.transpose(3, 1, 0, 2)       # in, s, h, i
            w4 = w4.reshape(2, 2, 128, 2, 128)  # kp, e, p, s, col
            w4 = w4.transpose(0, 2, 1, 3, 4)    # kp, p, e, s, col
            return f8v(w4)

        def prep_b(bvec):
            b4 = bvec.reshape(4, 2, 32)         # h, s, i
            b4 = b4.transpose(1, 0, 2).reshape(1, 256)  # [1, (s, 32h+i)]
            return bfv(b4)

        in_maps.append({
            "xq": f8v(q[b].reshape(4, 128, L)),
            "xk": f8v(k[b].reshape(4, 128, L)),
            "xv": bfv(v[b].reshape(4, 128, L)),
            "wq": prep_qk(wq_sl),
            "wk": prep_qk(wk_sl),
            "wv": bfv(Wv[sl].T.reshape(4, 128, 256)),
            "wo": bfv(Wout[:, sl].T.reshape(2, 128, 512)),
            "bq": prep_b(bq[sl] * 16.0),
            "bk": prep_b(bk[sl] * 16.0),
            "bv": bfv(bv[sl].reshape(1, 256)),
            "bo": np.ascontiguousarray(bout.reshape(4, 128, 1)) if g == 0
                  else np.zeros((4, 128, 1), np.float32),
            "ident": bfv(np.eye(128, dtype=np.float32)),
        })

    res = run_bass_kernel_spmd(_NC, in_maps, list(range(8)))
    LAST_RESULTS = res
    y = np.empty((4, 512, L), np.float32)
    for b in range(4):
        p0 = np.asarray(res.results[2 * b]["out"]).astype(np.float32)
        p1 = np.asarray(res.results[2 * b + 1]["out"]).astype(np.float32)
        y[b] = p0 + p1
    return y
